# revision 2
# baseline (speedup 1.0000x reference)
"""Trainium2 Bass kernel for nn_ErrorSimulator (fault_injection_batch_v2).

out = inputs * masks[random_indexes] + injection_sites[random_indexes]

Strategy (data-parallel over batch, 8 cores):
  - Each core owns B/8 = 8 samples of `inputs` plus a replicated copy of
    both lookup tables.
  - All streamed data is converted to bfloat16 on the host: the kernel is
    purely HBM-bandwidth-bound (gather + FMA + store), so halving the
    byte width halves device time.  The rel-err budget (2e-2) dwarfs the
    bf16 quantization error (~2.5e-3 measured).
  - A "chunk" packs SPC samples into one [128, E] SBUF tile.  The table
    gather is an indirect (SWDGE) DMA over the table viewed as
    [256*RPS, E], with per-partition row index idx[sample]*RPS + subrow.
  - Per chunk: load x, gather mask, gather site (3 concurrent DMA
    streams), then mul (DVE) + add (Pool), then store.  Memory-bound:
    8 MB of HBM traffic per core -> ~23 us at ~360 GB/s/core.
"""

import numpy as np
import ml_dtypes

import concourse.bass as bass
import concourse.mybir as mybir
import concourse.tile as tile
from concourse.bass_utils import run_bass_kernel_spmd

BF16 = ml_dtypes.bfloat16

# Problem shapes (hardcoded; see spec)
B, H, Wd, C = 64, 32, 32, 128
NSITES = 256
FEAT = H * Wd * C            # 131072 elems per sample
N_CORES = 8
BPC = B // N_CORES           # 8 samples per core

SPC = 2                      # samples per [128, E] chunk
N_CHUNKS = BPC // SPC        # chunks per core
RPS = 128 // SPC             # partition sub-rows per sample
E = FEAT // RPS              # elems per sub-row
NROWS = NSITES * RPS         # rows of the gathered table view
P = 128

SBUF_BUFS = 6
FUSE_SITE_ADD_INTO_DMA = False  # option A: cce add during site gather


def split_multi_waits(nc: bass.Bass) -> None:
    """The CoreV3 ISA encodes at most one sync-wait per instruction, but the
    Tile scheduler embeds one wait per dependency.  Hoist all but the last
    wait of each instruction onto same-engine NoOps placed directly before
    it (the sequencer stalls on each in program order, so semantics are
    unchanged)."""
    ctr = 0
    for f in nc.m.functions:
        for bb in f.blocks:
            insts = bb.instructions
            out = []
            changed = False
            for inst in insts:
                si = inst.sync_info
                waits = list(si.on_wait) if (si is not None and si.on_wait) else []
                if len(waits) > 1:
                    changed = True
                    for w in waits[:-1]:
                        ctr += 1
                        nop = mybir.InstNoOp(name=f"{inst.name}-hw{ctr}")
                        nop.engine = inst.engine
                        nop.sync_info = mybir.SyncInfo(on_wait=[w], on_update=[])
                        out.append(nop)
                    inst.sync_info = mybir.SyncInfo(
                        on_wait=[waits[-1]], on_update=list(si.on_update or [])
                    )
                out.append(inst)
            if changed:
                bb.instructions = out


def build_kernel(
    reps: int = 1,
    spc: int = SPC,
    bufs: int = SBUF_BUFS,
    fuse_site: bool = FUSE_SITE_ADD_INTO_DMA,
    mode: str = "full",  # full | direct (plain loads, wrong results) | copy | copy2
    store_engine: str = "sync",  # sync | scalar (second HWDGE ring)
    swdge_queues: int = 1,
    add_engine: str = "pool",  # pool | vector
) -> bass.Bass:
    n_chunks = BPC // spc
    rps = 128 // spc
    e = FEAT // rps
    nrows = NSITES * rps
    dt = mybir.dt.bfloat16

    nc = bass.Bass(num_swdge_queues=swdge_queues)
    x = nc.dram_tensor("x", [n_chunks, P, e], dt, kind="ExternalInput")
    sites = nc.dram_tensor("sites", [nrows, e], dt, kind="ExternalInput")
    masks = nc.dram_tensor("masks", [nrows, e], dt, kind="ExternalInput")
    offs = nc.dram_tensor("offs", [P, n_chunks], mybir.dt.int32, kind="ExternalInput")
    y = nc.dram_tensor("y", [n_chunks, P, e], dt, kind="ExternalOutput")

    with tile.TileContext(nc) as tc:
        with (
            tc.tile_pool(name="sbuf", bufs=bufs) as pool,
            tc.tile_pool(name="small", bufs=1) as spool,
        ):
            offs_tile = spool.tile([P, n_chunks], mybir.dt.int32)
            nc.sync.dma_start(out=offs_tile[:], in_=offs[:])
            for c in [c for _ in range(reps) for c in range(n_chunks)]:
                st = nc.scalar if store_engine == "scalar" else nc.sync
                x_t = pool.tile([P, e], dt, tag="x")
                nc.sync.dma_start(out=x_t[:], in_=x[c, :, :])
                if mode in ("copy", "copy2"):
                    eng = st if mode == "copy2" else nc.sync
                    eng.dma_start(out=y[c, :, :], in_=x_t[:])
                    continue
                m_t = pool.tile([P, e], dt, tag="m")
                if mode == "direct":
                    nc.gpsimd.dma_start(out=m_t[:], in_=masks[0:P, :])
                else:
                    nc.gpsimd.indirect_dma_start(
                        out=m_t[:],
                        out_offset=None,
                        in_=masks[:],
                        in_offset=bass.IndirectOffsetOnAxis(
                            ap=offs_tile[:, c : c + 1], axis=0
                        ),
                    )
                if mode != "nodve":
                    nc.vector.tensor_mul(out=x_t[:], in0=x_t[:], in1=m_t[:])
                if fuse_site:
                    nc.gpsimd.indirect_dma_start(
                        out=x_t[:],
                        out_offset=None,
                        in_=sites[:],
                        in_offset=bass.IndirectOffsetOnAxis(
                            ap=offs_tile[:, c : c + 1], axis=0
                        ),
                        compute_op=mybir.AluOpType.add,
                    )
                else:
                    s_t = pool.tile([P, e], dt, tag="s")
                    if mode == "direct":
                        nc.gpsimd.dma_start(out=s_t[:], in_=sites[0:P, :])
                    else:
                        nc.gpsimd.indirect_dma_start(
                            out=s_t[:],
                            out_offset=None,
                            in_=sites[:],
                            in_offset=bass.IndirectOffsetOnAxis(
                                ap=offs_tile[:, c : c + 1], axis=0
                            ),
                        )
                    if mode != "nodve":
                        add_eng = nc.gpsimd if add_engine == "pool" else nc.vector
                        add_eng.tensor_add(out=x_t[:], in0=x_t[:], in1=s_t[:])
                st.dma_start(out=y[c, :, :], in_=x_t[:])
    split_multi_waits(nc)
    return nc


_nc_cache = None


def _get_nc() -> bass.Bass:
    global _nc_cache
    if _nc_cache is None:
        _nc_cache = build_kernel()
    return _nc_cache


def _make_in_maps(inputs, injection_sites, masks, random_indexes, spc=SPC):
    n_chunks = BPC // spc
    rps = 128 // spc
    e = FEAT // rps
    nrows = NSITES * rps

    x_all = np.asarray(inputs).astype(BF16).reshape(B, FEAT)
    sites_r = np.asarray(injection_sites).astype(BF16).reshape(nrows, e)
    masks_r = np.asarray(masks).astype(BF16).reshape(nrows, e)
    idx = np.asarray(random_indexes, dtype=np.int32)

    p = np.arange(P)
    in_maps = []
    for k in range(N_CORES):
        idx_k = idx[k * BPC : (k + 1) * BPC].astype(np.int64)
        offs = np.empty((P, n_chunks), np.int32)
        for c in range(n_chunks):
            offs[:, c] = idx_k[c * spc + p // rps] * rps + p % rps
        in_maps.append(
            {
                "x": x_all[k * BPC : (k + 1) * BPC].reshape(n_chunks, P, e),
                "sites": sites_r,
                "masks": masks_r,
                "offs": offs.copy(),
            }
        )
    return in_maps


def run(inputs, injection_sites, masks, random_indexes, **spmd_kwargs):
    """Run the kernel; returns (output, BassKernelResults)."""
    in_maps = _make_in_maps(inputs, injection_sites, masks, random_indexes)
    res = run_bass_kernel_spmd(
        _get_nc(), in_maps, core_ids=list(range(N_CORES)), **spmd_kwargs
    )
    out = np.concatenate(
        [r["y"].reshape(BPC, FEAT).astype(np.float32) for r in res.results], axis=0
    )
    return out.reshape(B, H, Wd, C), res


def kernel(inputs, injection_sites, masks, random_indexes):
    out, _ = run(inputs, injection_sites, masks, random_indexes)
    return out
